# revision 42
# baseline (speedup 1.0000x reference)
"""Trainium2 Bass kernel for ItemEmbeddingLayer (embedding_lookup).

Reference computation:
    out = Q_matrix[items] @ skill_embedding[user]      # [8192, 128] f32

Sharding (the hint's data-parallel option): Q_matrix and the user's
embedding row are replicated; `items` is sharded batch-wise, 1024/core.

Per-core device kernel (final, ~27.7us vs 29.9us baseline):
  1. 8x indirect_dma_start gathers (128 offsets each, one per dest
     partition — the only offset-table shape the HW ucode supports) pull
     the 1024 needed Q rows (bf16 — exact, Q is binary).
  2. PE transposes flip each [128,128] block into [skill, item] layout;
     DVE/ACT alternate on the PSUM->SBUF copy-back.
  3. The matmul runs emb-stationary with the transposed output
     out^T[k, item]: lhsT = emb[s-chunk] (bf16 single pass; ~1.7e-3 rel
     from rounding emb, gate is 2e-2), rhs = the transposed Q rows.
     One [128, 128] PSUM bank per item chunk, 2 accumulating matmuls.
  4. PSUM -> SBUF copies cast to bf16 (DVE/ACT alternate); output DMAs
     drain finished banks while later chunks compute, with a small
     final DMA on the critical path. The host transposes each core's
     [128, 1024] block back to [1024, 128] f32 (pure layout unshard;
     bf16 output rounding adds ~2e-3 rel).
"""

import numpy as np
import ml_dtypes

import concourse.bass as bass
import concourse.bacc as bacc
import concourse.mybir as mybir
from concourse.tile import TileContext
from concourse.bass_utils import run_bass_kernel_spmd

N_CORES = 8
L = 8192            # total items (seq len)
LC = L // N_CORES   # items per core
S = 256             # skills
K = 128             # hidden
R = 4096            # Q_matrix rows (item vocab)
P = 128             # partitions
NCH = LC // P       # 128-item chunks per core (8)
NB = 8              # PSUM output banks ([128, 128] each)


def build_bass() -> bass.Bass:
    nc = bacc.Bacc(trn_type="TRN2", dynamic_dma_scratch_size=32768)
    q = nc.declare_dram_parameter("q_bf16", [R, S], mybir.dt.bfloat16, isOutput=False)
    idx = nc.declare_dram_parameter("idx", [P, NCH], mybir.dt.int32, isOutput=False)
    # wgt packs [ident | emb_j0 | emb_j1] so one DMA loads all matmul weights
    wgt = nc.declare_dram_parameter("wgt", [P, P + 2 * K], mybir.dt.bfloat16, isOutput=False)
    outT = nc.declare_dram_parameter("outT", [P, LC], mybir.dt.bfloat16, isOutput=True)

    CPB = NCH // NB   # chunks per psum bank

    with (
        TileContext(nc) as tc,
        tc.tile_pool(name="main", bufs=1) as pool,
        tc.tile_pool(name="tps", bufs=4, space="PSUM") as tpsum,
        tc.tile_pool(name="acc", bufs=4, space="PSUM") as apsum,
    ):
        idx_t = pool.tile([P, NCH], mybir.dt.int32)
        nc.sync.dma_start(out=idx_t[:], in_=idx[:])

        # q_sb[c][p, s] = Q[items[c*128 + p], s]; one 128-offset SWDGE
        # instruction per chunk. 128 offsets (one per dest partition) is
        # the hardware ucode's limit; batched offset tables scramble, and
        # dma_gather's bulk ucode is slower per row. ~1.4us each is the
        # per-item-gather floor on this hardware.
        q_sb = [
            pool.tile([P, S], mybir.dt.bfloat16, name=f"q_sb{c}")
            for c in range(NCH)
        ]
        for c in range(NCH):
            nc.gpsimd.indirect_dma_start(
                out=q_sb[c][:],
                out_offset=None,
                in_=q[:],
                in_offset=bass.IndirectOffsetOnAxis(ap=idx_t[:, c : c + 1], axis=0),
            )

        wgt_t = pool.tile([P, P + 2 * K], mybir.dt.bfloat16)
        nc.sync.dma_start(out=wgt_t[:], in_=wgt[:])
        ident_t = wgt_t[:, 0:P]
        emb_j = [wgt_t[:, P + j * K : P + (j + 1) * K] for j in range(2)]

        # qT[p, j, i] = Q[items[i], j*128 + p]   (matmul rhs layout)
        qT = pool.tile([P, 2, LC], mybir.dt.bfloat16)
        occ = pool.tile([P, LC], mybir.dt.bfloat16)

        for n in range(NB):
            for cc in range(CPB):
                c = n * CPB + cc
                for j in range(2):
                    tp = tpsum.tile([P, P], mybir.dt.bfloat16, tag="tp")
                    nc.tensor.transpose(
                        tp[:], q_sb[c][:, j * P : (j + 1) * P], ident_t
                    )
                    dst = qT[:, j, c * P : (c + 1) * P]
                    if j == 0:
                        nc.vector.tensor_copy(dst, tp[:])
                    else:
                        nc.scalar.copy(dst, tp[:])

            NW = CPB * P  # columns per bank
            ps = apsum.tile([P, NW], mybir.dt.float32, tag="ps")
            for j in range(2):
                nc.tensor.matmul(
                    ps[:],
                    emb_j[j],
                    qT[:, j, n * NW : (n + 1) * NW],
                    start=(j == 0),
                    stop=(j == 1),
                )
            # odd banks (incl. the last, critical-path one) on DVE — its
            # PSUM copy is ~140ns faster than ACT's
            if n % 2 == 0:
                nc.scalar.copy(occ[:, n * NW : (n + 1) * NW], ps[:])
            else:
                nc.vector.tensor_copy(occ[:, n * NW : (n + 1) * NW], ps[:])

            # drain finished banks while later chunks still compute;
            # staggered boundaries keep the last (critical-path) DMA small,
            # and it issues from ACT so it never queues behind Sync's
            # earlier ~0.6us dma_start issue
            bounds = {3: 0, 5: 4, 7: 6}
            if n in bounds:
                d0 = bounds[n] * NW
                d1 = (n + 1) * NW
                eng = nc.scalar if n == NB - 1 else nc.sync
                eng.dma_start(out=outT[:, d0:d1], in_=occ[:, d0:d1])

    nc.compile()
    return nc


_CACHE: dict = {}


def get_nc() -> bass.Bass:
    if "nc" not in _CACHE:
        _CACHE["nc"] = build_bass()
    return _CACHE["nc"]


def make_in_maps(user, Q_matrix, items, skill_embedding):
    user = int(np.asarray(user))
    Q = np.asarray(Q_matrix, dtype=np.float32)
    items = np.asarray(items).astype(np.int64)
    emb32 = np.ascontiguousarray(np.asarray(skill_embedding)[user], dtype=np.float32)

    q_bf = Q.astype(ml_dtypes.bfloat16)  # exact: Q is 0/1
    emb_t = emb32.reshape(2, P, K).transpose(1, 0, 2).reshape(P, 2 * K)
    ident = np.eye(P, dtype=np.float32)
    wgt = np.ascontiguousarray(
        np.concatenate([ident, emb_t], axis=1).astype(ml_dtypes.bfloat16)
    )

    in_maps = []
    for i in range(N_CORES):
        it = items[i * LC : (i + 1) * LC].astype(np.int32)
        # gather c pulls row idx[p, c] into partition p
        idx_arr = np.ascontiguousarray(it.reshape(NCH, P).T)  # [128, NCH]
        in_maps.append({"q_bf16": q_bf, "idx": idx_arr, "wgt": wgt})
    return in_maps


def kernel(user, Q_matrix, items, skill_embedding, _trace=False, _result_box=None):
    in_maps = make_in_maps(user, Q_matrix, items, skill_embedding)
    res = run_bass_kernel_spmd(get_nc(), in_maps, list(range(N_CORES)), trace=_trace)
    if _result_box is not None:
        _result_box.append(res)
    out = np.concatenate(
        [
            np.ascontiguousarray(res.results[i]["outT"].astype(np.float32).T)
            for i in range(N_CORES)
        ],
        axis=0,
    )
    return np.ascontiguousarray(out, dtype=np.float32)
